# revision 26
# baseline (speedup 1.0000x reference)
"""Trainium2 Bass kernel for nn_EncoderBlock (conformer-style encoder block).

Model: x+posenc -> 4x [LN -> depthwise conv(K=7) -> pointwise HxH -> ReLU -> +res]
       -> [LN -> MHA(8 heads, d=64) -> +res] -> [LN -> HxH -> ReLU -> +res]
B=32, L=400, H=512.

Sharding: pure data-parallel over batch. 8 cores x 4 batches, weights
replicated, no collectives.

bf16 datapath: activations, weights, and norm tiles are bf16 (DVE
tensor_tensor in 2x mode, copies 4x; matmuls full-rate with fp32 PSUM).
LN gamma/beta fold into downstream weights host-side; the per-token LN shift
(-m*s) rides into every matmul as a K=1/K=2 accumulation against a
[shift-row; ones-row] pair, so the materialized norm is just y = x*s — one
wide tensor_tensor per batch (all 4 channel groups live in one padded
[128, 4*LP] tile). Depthwise-conv taps run on the PE as 7 accumulating
matmuls per group against host-built diagonal weight matrices; the LN-shift
contribution enters the pointwise PSUM via a K=7 matmul against 7 shifted
copies of the shift row (built by tiny SWDGE sbuf->sbuf DMAs on the idle
GPSIMD queue). PSUM->SBUF moves ride ACT (ReLU/bias fused) except in the
ACT-heavy attention phase, where the softmax normalize runs on DVE reading
the AV PSUM directly. Weights load as one batched multi-dim-AP DMA per
matrix, queued behind the activation loads so compute starts immediately.
Softmax uses KC=100 key chunks (paired exp needs no memset) and gets
sum(exp) free from a ones column appended to V^T.
"""

import math
import sys

import numpy as np

for _p in ("/opt/trn_rl_repo",):
    if _p not in sys.path:
        sys.path.append(_p)

B, L, H, NH, K, NC = 32, 400, 512, 8, 7, 4
D = H // NH          # 64
NCORES = 8
BPC = B // NCORES    # batches per core
G = H // 128         # 4 feature groups
KC = 100             # key chunk of L=400 (4 chunks)
SR = 97              # stats tile rows; batch b's row = 32*b
PAD = K // 2         # 3
LP = L + 8           # padded slot width: y data at col 4
EPS = 1e-5
H3 = 3 * H

_cache = {}


def _bf16(a):
    import ml_dtypes
    return np.asarray(a, np.float32).astype(ml_dtypes.bfloat16)


def _pos_encoding():
    pos = np.arange(L, dtype=np.float64)[:, None]
    div = np.exp(np.arange(0, H, 2, dtype=np.float64) * (-math.log(10000.0) / H))
    pe = np.zeros((L, H), np.float64)
    pe[:, 0::2] = np.sin(pos * div)
    pe[:, 1::2] = np.cos(pos * div)
    return pe.astype(np.float32)


def _install_drain_patch(tile_mod):
    """This walrus build rejects >2 sem waits on one control instruction
    ("Too many sync wait commands"). TileContext's closing drain carries the
    whole global clock; spread it over 1-wait nops instead."""
    from concourse.vector_clock import ScopedClock, VectorClock

    def _patched_drain_and_barrier(self, tick_clock, wait_clock):
        gc = tick_clock.global_clock
        n = len(gc)
        for i in range(n):
            t = gc[i]
            if t > 0:
                nop = self.nc.sync.nop()
                vec = [0] * n
                vec[i] = t
                wait_clock.add_sem_waits(nop.ins, ScopedClock({None: VectorClock(vec)}))
        self.nc.sync.drain()
        self.nc.all_engine_barrier()
        assert self.sems is not None
        popped = self.nc._tile_sem_poison_stack.pop()
        assert popped is self._sem_poison
        self.nc.clear_and_free_semaphores(list(self.sems.allocated().values()))
        self.nc.all_engine_barrier()

    tile_mod.TileContext._drain_and_barrier = _patched_drain_and_barrier


def _split_excess_waits(nc, mybir, limit=1):
    """This walrus build caps semaphore waits per instruction. Move excess
    waits onto same-engine InstNoOp carriers placed just before the
    instruction (engines execute their stream in order, so ordering is
    preserved)."""
    import bass_rust
    for f in nc.m.functions:
        for blk in f.blocks:
            out = []
            changed = False
            for inst in blk.instructions:
                si = inst.sync_info
                waits = list(si.on_wait) if si is not None and si.on_wait else []
                if len(waits) > limit:
                    head, keep = waits[:-limit], waits[-limit:]
                    for j, w in enumerate(head):
                        nop = bass_rust.InstNoOp(
                            name=f"{inst.name}_w{j}", ins=[], outs=[])
                        nop.engine = inst.engine
                        nop.sync_info = mybir.SyncInfo(on_wait=[w],
                                                       on_update=[])
                        nc.register_instruction(nop)
                        out.append(nop)
                    si.on_wait = keep
                    changed = True
                out.append(inst)
            if changed:
                blk.instructions[:] = out


def _build_program(mask_all_ones, repeat=1):
    import concourse.bass as bass
    import concourse.mybir as mybir
    import concourse.tile as tile

    _install_drain_patch(tile)

    f32 = mybir.dt.float32
    f32r = mybir.dt.float32r
    bf16 = mybir.dt.bfloat16
    AF = mybir.ActivationFunctionType
    OP = mybir.AluOpType

    nc = bass.Bass("TRN2", target_bir_lowering=False, debug=False,
                   num_devices=NCORES)

    # ---------------- DRAM parameters (per core) ----------------
    xT = nc.dram_tensor("xT", [BPC, 128, G * L], bf16, kind="ExternalInput").ap()
    wpw = nc.dram_tensor("wpw", [NC, 128, G * H], bf16, kind="ExternalInput").ap()
    dwdiag = nc.dram_tensor("dwdiag", [NC, 128, K * G * 128], bf16,
                            kind="ExternalInput").ap()
    wj = nc.dram_tensor("wj", [K, NC * H], f32r, kind="ExternalInput").ap()
    pwb = nc.dram_tensor("pwb", [128, NC * G], f32, kind="ExternalInput").ap()
    wqkv = nc.dram_tensor("wqkv", [128, G * H3], bf16, kind="ExternalInput").ap()
    csqkv = nc.dram_tensor("csqkv", [2, 2 * H], f32r, kind="ExternalInput").ap()
    csv = nc.dram_tensor("csv", [2, H], f32r, kind="ExternalInput").ap()
    wout = nc.dram_tensor("wout", [128, G * H], bf16, kind="ExternalInput").ap()
    bout = nc.dram_tensor("bout", [128, G], f32, kind="ExternalInput").ap()
    wff = nc.dram_tensor("wff", [128, G * H], bf16, kind="ExternalInput").ap()
    csff = nc.dram_tensor("csff", [1, H], f32r, kind="ExternalInput").ap()
    bff = nc.dram_tensor("bff", [128, G], f32, kind="ExternalInput").ap()
    eb4 = nc.dram_tensor("eb4", [128, BPC * 64], bf16, kind="ExternalInput").ap()
    ones1 = nc.dram_tensor("ones1", [1, 128], f32r, kind="ExternalInput").ap()
    ones1f = nc.dram_tensor("ones1f", [1, 64], f32r, kind="ExternalInput").ap()
    nrowi = nc.dram_tensor("nrowi", [2, LP], f32r, kind="ExternalInput").ap()
    voinit = nc.dram_tensor("voinit", [128, NH * (D + 1)], bf16,
                            kind="ExternalInput").ap()
    mb = nc.dram_tensor("mb", [128, BPC * G], f32, kind="ExternalInput").ap()
    outT = nc.dram_tensor("outT", [BPC, 128, G * L], bf16,
                          kind="ExternalOutput").ap()

    with tile.TileContext(nc) as tc:
        with (
            tc.tile_pool(name="const", bufs=1) as constp,
            tc.tile_pool(name="xp", bufs=1) as xp,
            tc.tile_pool(name="wp", bufs=1) as wp,
            tc.tile_pool(name="scr", bufs=2) as scr,
            tc.tile_pool(name="attp", bufs=1) as attp,
            tc.tile_pool(name="exptp", bufs=2) as exptp,
            tc.tile_pool(name="ps", bufs=2, space="PSUM") as ps,
        ):
            c_zero = constp.tile([128, 1], f32, tag="c_zero")
            nc.vector.memset(c_zero[:], 0.0)
            c_eps = constp.tile([128, 1], f32, tag="c_eps")
            nc.vector.memset(c_eps[:], EPS)
            nc.const_aps.aps[(f32, 0.0)] = c_zero[:]
            nc.const_aps.aps[(f32, EPS)] = c_eps[:]

            def const_tile(shape, src, nm, dt=f32):
                t = constp.tile(shape, dt, tag=nm, name=nm)
                nc.sync.dma_start(t[:], src)
                return t

            # y tiles: one [128, G*LP] padded tile per batch parity; data for
            # group g at cols [g*LP+4, g*LP+4+L). Margins zeroed once.
            y4 = [constp.tile([128, G * LP], bf16, tag=f"y4_{p}",
                              name=f"y4_{p}") for p in range(2)]
            y4o = [constp.tile([128, G * LP], bf16, tag=f"y4o_{p}",
                               name=f"y4o_{p}") for p in range(2)]
            for p in range(2):
                nc.vector.memset(y4[p][:], 0.0)
                nc.vector.memset(y4o[p][:], 0.0)
            # per-batch shift/ones row pairs: row0 = -m*s (data at col 4),
            # row1 = ones over the data region
            nrow2 = [constp.tile([2, LP], f32r, tag=f"nrow2_{b}",
                                 name=f"nrow2_{b}")
                     for b in range(BPC)]
            for b in range(BPC):
                nc.sync.dma_start(nrow2[b][:], nrowi[:])
            # persistent v^T tiles with ones column pre-set
            vo = [[constp.tile([128, NH * (D + 1)], bf16,
                               tag=f"vo_{c}_{p}", name=f"vo_{c}_{p}")
                   for p in range(2)] for c in range(G)]
            for c in range(G):
                for p in range(2):
                    nc.sync.dma_start(vo[c][p][:], voinit[:])

            for _rep in range(repeat):
                # -------- activation loads first, then consts, then weights
                # in use order (single HWDGE queue drains in issue order) ----
                xt = []      # per-batch [128, G*L] residual tiles
                for b in range(BPC):
                    t = xp.tile([128, G * L], bf16, tag=f"x_{b}", name=f"x_{b}")
                    nc.sync.dma_start(t[:], xT[b])
                    xt.append(t)

                c_eb4 = const_tile([128, BPC * 64], eb4[:], 'eb4', bf16)
                c_ones1 = const_tile([1, 128], ones1[:], 'ones1', f32r)
                c_ones1f = const_tile([1, 64], ones1f[:], 'ones1f', f32r)
                c_wj = const_tile([K, NC * H], wj[:], 'wj', f32r)
                c_pwb = const_tile([128, NC * G], pwb[:], 'pwb')

                w_pw_all, w_dg = [], []
                for i in range(NC):
                    t = wp.tile([128, G * H], bf16, tag=f"pw_{i}", name=f"pw_{i}")
                    nc.sync.dma_start(t[:], wpw[i])
                    w_pw_all.append(t)
                    t = wp.tile([128, K * G * 128], bf16, tag=f"dg_{i}",
                                name=f"dg_{i}")
                    nc.sync.dma_start(t[:], dwdiag[i])
                    w_dg.append(t)

                w_qkv = wp.tile([128, G * H3], bf16, tag="qkv", name="qkv")
                nc.sync.dma_start(w_qkv[:], wqkv[:])
                c_csqkv = const_tile([2, 2 * H], csqkv[:], 'csqkv', f32r)
                c_csv = const_tile([2, H], csv[:], 'csv', f32r)
                w_out = wp.tile([128, G * H], bf16, tag="out", name="out")
                nc.sync.dma_start(w_out[:], wout[:])
                c_bout = const_tile([128, G], bout[:], 'bout')
                w_ff = wp.tile([128, G * H], bf16, tag="ff", name="ff")
                nc.sync.dma_start(w_ff[:], wff[:])
                c_csff = const_tile([1, H], csff[:], 'csff', f32r)
                c_bff = const_tile([128, G], bff[:], 'bff')
                c_mb = None if mask_all_ones else const_tile(
                    [128, BPC * G], mb[:], 'mb')

                def xsl(b, g):
                    return xt[b][:, L * g:L * (g + 1)]

                def ysl(p, g, j):
                    """y view of group g shifted for tap j (x-hat[l+j-3]),
                    always 4-byte aligned: odd j reads y4 (data at col 4),
                    even j reads the one-right-shifted copy y4o (col 5)."""
                    if j % 2 == 1:
                        return y4[p][:, LP * g + j + 1:LP * g + j + 1 + L]
                    return y4o[p][:, LP * g + j + 2:LP * g + j + 2 + L]

                # ---------- layer norm ----------
                def layer_norm(tag, consume, sq_dve=False):
                    """LN over H, batch-pipelined: each batch gets its own
                    [2, L] stats PSUM (row0 = E[x], row1 = E[x^2]) so batch
                    b's consume starts as soon as its own 8 stats matmuls
                    and short fp32 chain finish. ACT Exp writes the bf16 s
                    row directly; the shift row -m*s lands straight in
                    nrow2[b]."""
                    with tc.tile_pool(name=f"pst_{tag}", bufs=1,
                                      space="PSUM") as pst:
                        for b in range(BPC):
                            p = b % 2
                            ebT = c_eb4[:, 64 * b:64 * (b + 1)]
                            ex1 = pst.tile([64, L], f32, tag="st1",
                                           name=f"ex1_{tag}_{b}")
                            ex2 = pst.tile([64, L], f32, tag="st2",
                                           name=f"ex2_{tag}_{b}")
                            for g in range(G):
                                nc.tensor.matmul(ex1[:], ebT, xsl(b, g),
                                                 start=(g == 0),
                                                 stop=(g == G - 1))
                            sq = scr.tile([128, G * L], bf16, tag="sq",
                                          name=f"sq_{tag}_{b}")
                            if sq_dve:
                                nc.vector.tensor_tensor(sq[:], xt[b][:],
                                                        xt[b][:], op=OP.mult)
                            else:
                                nc.scalar.activation(sq[:], xt[b][:], AF.Square)
                            for g in range(G):
                                nc.tensor.matmul(ex2[:], ebT,
                                                 sq[:, L * g:L * (g + 1)],
                                                 start=(g == 0),
                                                 stop=(g == G - 1))
                            # (DVE may read only one PSUM operand per op)
                            m_sb = scr.tile([1, L], f32, tag="ln_m", bufs=3,
                                            name=f"m_{tag}_{b}")
                            nc.vector.tensor_copy(m_sb[:], ex1[0:1, :])
                            msq = scr.tile([1, L], f32, tag="ln_tmp", bufs=3,
                                           name=f"msq_{tag}_{b}")
                            nc.vector.tensor_tensor(msq[:], m_sb[:],
                                                    ex1[0:1, :], op=OP.mult)
                            var = scr.tile([1, L], f32, tag="ln_tmp", bufs=3,
                                           name=f"var_{tag}_{b}")
                            nc.vector.scalar_tensor_tensor(
                                var[:], msq[:], -1.0, ex2[0:1, :],
                                op0=OP.mult, op1=OP.add)
                            lnv = scr.tile([1, L], f32, tag="ln_tmp", bufs=3,
                                           name=f"lnv_{tag}_{b}")
                            nc.scalar.activation(lnv[:], var[:], AF.Ln, bias=EPS)
                            s1 = scr.tile([1, L], f32r, tag="ln_s1", bufs=4,
                                          name=f"s1_{tag}_{b}")
                            nc.scalar.activation(s1[:], lnv[:], AF.Exp,
                                                 scale=-0.5)
                            nc.vector.scalar_tensor_tensor(
                                nrow2[b][0:1, 4:4 + L], m_sb[:], -1.0, s1[:],
                                op0=OP.mult, op1=OP.mult)
                            sbp = ps.tile([128, L], f32, tag="mm",
                                          name=f"sb_{tag}_{b}")
                            nc.tensor.matmul(sbp[:], c_ones1[:], s1[:],
                                             start=True, stop=True)
                            sb = scr.tile([128, L], bf16, tag="sb", bufs=2,
                                          name=f"sbs_{tag}_{b}")
                            nc.scalar.activation(sb[:], sbp[:], AF.Copy)
                            for g in range(G):
                                nc.vector.tensor_tensor(
                                    y4[p][:, LP * g + 4:LP * g + 4 + L],
                                    xsl(b, g), sb[:], op=OP.mult)
                            consume(b, p)

                # ================= conv blocks =================
                for i in range(NC):
                    def conv_consume(b, p, i=i):
                        # one-right-shifted copy of y for the even taps
                        nc.sync.dma_start(
                            y4o[p][:].rearrange("q (g l) -> q g l", l=LP)
                            [:, :, 5:5 + L],
                            y4[p][:].rearrange("q (g l) -> q g l", l=LP)
                            [:, :, 4:4 + L])
                        # shifted copies of the -m*s row for the K=7 shift-
                        # correction matmul (reads padded source)
                        nt = scr.tile([K, L], f32r, tag="nshift", bufs=2,
                                      name=f"nt_{i}_{b}")
                        for j in range(K):
                            nc.sync.dma_start(nt[j:j + 1, :],
                                              nrow2[b][0:1, j + 1:j + 1 + L])
                        acc = scr.tile([128, G * L], bf16, tag="acc", bufs=2,
                                       name=f"acc_{i}_{b}")
                        with tc.tile_pool(name=f"ptap_{i}_{b}", bufs=2,
                                          space="PSUM") as ptap, \
                             tc.tile_pool(name=f"pmc_{i}_{b}", bufs=2,
                                          space="PSUM") as pmc:
                            for g in range(G):
                                pt = ptap.tile([128, L], f32, tag="tap",
                                               name=f"pt_{i}_{b}_{g}")
                                for j in range(K):
                                    nc.tensor.matmul(
                                        pt[:],
                                        w_dg[i][:, (j * G + g) * 128:
                                                (j * G + g + 1) * 128],
                                        ysl(p, g, j),
                                        start=(j == 0), stop=(j == K - 1))
                                nc.scalar.activation(
                                    acc[:, L * g:L * (g + 1)], pt[:], AF.Copy)
                            cvall = scr.tile([128, G * L], bf16, tag="cv",
                                             name=f"cv_{i}_{b}")
                            for o in range(G):
                                pm = pmc.tile([128, L], f32, tag="pm",
                                              name=f"pm_{i}_{b}_{o}")
                                for g in range(G):
                                    nc.tensor.matmul(
                                        pm[:],
                                        w_pw_all[i][:, H * g + 128 * o:
                                                    H * g + 128 * (o + 1)],
                                        acc[:, L * g:L * (g + 1)],
                                        start=(g == 0), stop=False)
                                nc.tensor.matmul(
                                    pm[:],
                                    c_wj[:, i * H + 128 * o:i * H + 128 * (o + 1)],
                                    nt[:], start=False, stop=True)
                                nc.scalar.activation(
                                    cvall[:, L * o:L * (o + 1)], pm[:], AF.Relu,
                                    bias=c_pwb[:, i * G + o:i * G + o + 1])
                            nc.vector.tensor_tensor(xt[b][:], xt[b][:],
                                                    cvall[:], op=OP.add)

                    layer_norm(f"c{i}", conv_consume)

                # ================= attention =================
                def att_consume(b, p):
                    qsb, ksb = [], []
                    for m in range(2 * G):
                        pm = ps.tile([128, L], f32, tag="mm", name=f"pqk_{b}_{m}")
                        for g in range(G):
                            nc.tensor.matmul(
                                pm[:],
                                w_qkv[:, H3 * g + 128 * m:H3 * g + 128 * (m + 1)],
                                y4[p][:, LP * g + 4:LP * g + 4 + L],
                                start=(g == 0), stop=False)
                        nc.tensor.matmul(
                            pm[:], c_csqkv[:, 128 * m:128 * (m + 1)],
                            nrow2[b][0:2, 4:4 + L], start=False, stop=True)
                        t = attp.tile([128, L], bf16, tag=f"qk_{m}",
                                      bufs=2, name=f"qk_{b}_{m}")
                        nc.vector.tensor_copy(t[:], pm[:])
                        (qsb if m < G else ksb).append(t)
                    # v pre-transposed with bias+shift via K=2 row pair;
                    # ones column pre-set gives sum(exp) free in AV
                    for c in range(G):
                        pv = ps.tile([128, H], f32, tag="mm", name=f"pv_{b}_{c}")
                        for g in range(G):
                            nc.tensor.matmul(
                                pv[0:KC, :],
                                y4[p][:, LP * g + 4 + KC * c:
                                      LP * g + 4 + KC * (c + 1)],
                                w_qkv[:, H3 * g + 2 * H:H3 * g + 3 * H],
                                start=(g == 0), stop=False)
                        nc.tensor.matmul(
                            pv[0:KC, :],
                            nrow2[b][0:2, 4 + KC * c:4 + KC * (c + 1)],
                            c_csv[:], start=False, stop=True)
                        vv = vo[c][p][:].rearrange("p (h d) -> p h d", d=D + 1)
                        pvv = pv[:, 0:H].rearrange("p (h d) -> p h d", d=D)
                        nc.vector.tensor_copy(vv[0:KC, :, 0:D], pvv[0:KC, :, :])
                    att = []
                    for g in range(G):
                        att.append(attp.tile([128, L], bf16, tag=f"ao_{g}",
                                             bufs=2, name=f"ao_{b}_{g}"))
                    with tc.tile_pool(name=f"psatt_{b}", bufs=2,
                                      space="PSUM") as psatt:
                        for h in range(NH):
                            g2, ho = h // 2, (h % 2) * 64
                            # scores^T [k, q], one 1-bank psum per key chunk
                            ex = exptp.tile([128, G * L], bf16, tag="expT",
                                            bufs=3, name=f"ex_{b}_{h}")
                            for c in range(G):
                                sc = psatt.tile([128, 512], f32, tag="sc",
                                                name=f"sc_{b}_{h}_{c}")
                                nc.tensor.matmul(
                                    sc[0:KC, 0:L],
                                    ksb[g2][ho:ho + 64, KC * c:KC * (c + 1)],
                                    qsb[g2][ho:ho + 64, :],
                                    start=True, stop=True)
                                if mask_all_ones:
                                    nc.scalar.activation(
                                        ex[0:KC, L * c:L * c + L],
                                        sc[0:KC, 0:L], AF.Exp)
                                else:
                                    nc.scalar.activation(
                                        ex[0:KC, L * c:L * c + L],
                                        sc[0:KC, 0:L], AF.Exp,
                                        bias=c_mb[0:KC,
                                                  b * G + c:b * G + c + 1])
                            # AV with ones row -> [65, L]; row 64 = sum(exp)
                            av = psatt.tile([D + 1, L], f32, tag="av",
                                            name=f"av_{b}_{h}")
                            for c in range(G):
                                nc.tensor.matmul(
                                    av[:],
                                    vo[c][p][0:KC,
                                             (D + 1) * h:(D + 1) * (h + 1)],
                                    ex[0:KC, L * c:L * c + L],
                                    start=(c == 0), stop=(c == G - 1))
                            inv = scr.tile([1, L], f32r, tag="inv",
                                           name=f"inv_{b}_{h}")
                            with nc.allow_low_precision(reason="f32r feed"):
                                nc.vector.reciprocal(inv[:], av[D:D + 1, :])
                            # inv broadcast rides the sc tag rotation (slot
                            # freed once this head's exps are read)
                            ib = psatt.tile([64, 512], f32, tag="sc",
                                            name=f"ib_{b}_{h}")
                            nc.tensor.matmul(ib[:, 0:L], c_ones1f[:], inv[:],
                                             start=True, stop=True)
                            ibs = scr.tile([64, L], bf16, tag="ibs",
                                           name=f"ibs_{b}_{h}")
                            nc.vector.tensor_copy(ibs[:], ib[:, 0:L])
                            nc.vector.tensor_tensor(att[g2][ho:ho + 64, :],
                                                    av[0:D, :], ibs[:],
                                                    op=OP.mult)
                    # output projection + residual for this batch
                    aoall = scr.tile([128, G * L], bf16, tag="cv",
                                     name=f"aa_{b}")
                    for o in range(G):
                        pm = ps.tile([128, L], f32, tag="mm", name=f"po_{b}_{o}")
                        for g in range(G):
                            nc.tensor.matmul(
                                pm[:],
                                w_out[:, H * g + 128 * o:H * g + 128 * (o + 1)],
                                att[g][:], start=(g == 0), stop=(g == G - 1))
                        nc.vector.tensor_scalar_add(
                            aoall[:, L * o:L * (o + 1)], pm[:],
                            c_bout[:, o:o + 1])
                    nc.vector.tensor_tensor(xt[b][:], xt[b][:], aoall[:],
                                            op=OP.add)

                layer_norm("a", att_consume, sq_dve=True)

                # ================= feed-forward =================
                def ff_consume(b, p):
                    foall = scr.tile([128, G * L], bf16, tag="cv",
                                     name=f"fo_{b}")
                    for o in range(G):
                        pm = ps.tile([128, L], f32, tag="mm", name=f"pff_{b}_{o}")
                        for g in range(G):
                            nc.tensor.matmul(
                                pm[:],
                                w_ff[:, H * g + 128 * o:H * g + 128 * (o + 1)],
                                y4[p][:, LP * g + 4:LP * g + 4 + L],
                                start=(g == 0), stop=False)
                        nc.tensor.matmul(
                            pm[:], c_csff[:, 128 * o:128 * (o + 1)],
                            nrow2[b][0:1, 4:4 + L], start=False, stop=True)
                        nc.scalar.activation(foall[:, L * o:L * (o + 1)], pm[:],
                                             AF.Relu, bias=c_bff[:, o:o + 1])
                    # final residual add (bf16 store; host converts to f32)
                    xo = scr.tile([128, G * L], bf16, tag="xo", bufs=2,
                                  name=f"xo_{b}")
                    nc.vector.tensor_tensor(xo[:], xt[b][:], foall[:], op=OP.add)
                    nc.sync.dma_start(outT[b], xo[:])

                layer_norm("f", ff_consume, sq_dve=True)
    _split_excess_waits(nc, mybir, limit=1)
    return nc


def _prep_inputs(inputs):
    """Host-side packing shared by all cores."""
    x = np.asarray(inputs["x"], np.float32)
    mask = np.asarray(inputs["mask"])
    pe = _pos_encoding()
    xpe = np.ascontiguousarray((x + pe[None]).transpose(0, 2, 1))  # (B, H, L)

    conv_ln_g = np.asarray(inputs["conv_ln_g"], np.float32)
    conv_ln_b = np.asarray(inputs["conv_ln_b"], np.float32)
    dw_w = np.asarray(inputs["dw_w"], np.float32)[:, :, 0, :]  # (NC, H, K)
    dw_b = np.asarray(inputs["dw_b"], np.float32)
    pw_w = np.asarray(inputs["pw_w"], np.float32)        # (NC, H_out, H_in)
    pw_b = np.asarray(inputs["pw_b"], np.float32)
    att_ln_g = np.asarray(inputs["att_ln_g"], np.float32)
    att_ln_b = np.asarray(inputs["att_ln_b"], np.float32)
    qkv_w = np.asarray(inputs["qkv_w"], np.float32)      # (3H, H)
    qkv_b = np.asarray(inputs["qkv_b"], np.float32)
    out_w = np.asarray(inputs["out_w"], np.float32)
    out_b = np.asarray(inputs["out_b"], np.float32)
    ff_ln_g = np.asarray(inputs["ff_ln_g"], np.float32)
    ff_ln_b = np.asarray(inputs["ff_ln_b"], np.float32)
    ff_w = np.asarray(inputs["ff_w"], np.float32)
    ff_b = np.asarray(inputs["ff_b"], np.float32)

    def vecpack(v):  # (..., H) -> [128, n*G], col n*G-index = (outer, g)
        v = np.asarray(v, np.float32).reshape(-1, G, 128)
        return np.ascontiguousarray(v.transpose(2, 0, 1).reshape(128, -1))

    # ---- conv blocks: fold LN gamma into dw weights, beta into pw bias ----
    dwp = dw_w * conv_ln_g[:, :, None]                    # (NC, H, K) folded
    const_c = conv_ln_b * dw_w.sum(-1)                    # beta * sum_j dw
    eff_pwb = pw_b + np.einsum("ioc,ic->io", pw_w, dw_b + const_c)
    # K=7 shift-correction weights: WJ[i][j, o] = sum_c pw[o,c]*dwp[c,j]
    wj_all = np.einsum("ioc,icj->ijo", pw_w, dwp)         # (NC, K, H)
    wj_pack = np.ascontiguousarray(
        wj_all.transpose(1, 0, 2).reshape(K, NC * H))
    # diag matrices for the PE taps, pre-packed to the SBUF tile layout:
    # [i, p, (j*G+g)*128 + c] = dwp[i, 128g+p, j] if c == p else 0
    dwdiag = np.zeros((NC, 128, K * G * 128), np.float32)
    for i in range(NC):
        for j in range(K):
            for g in range(G):
                blk = (j * G + g) * 128
                dwdiag[i, np.arange(128), blk + np.arange(128)] = \
                    dwp[i, 128 * g:128 * (g + 1), j]

    # ---- attention: fold gamma into W rows, beta into bias; scale q ----
    scale = 1.0 / math.sqrt(D)
    wq = qkv_w * att_ln_g[None, :]
    bq = qkv_b + qkv_w @ att_ln_b
    wq[:H] *= scale
    bq[:H] *= scale
    cs = wq.sum(1)                                        # (3H,)
    csqkv = np.stack([cs[:2 * H], bq[:2 * H]], 0)          # (2, 2H)
    csv = np.stack([cs[2 * H:], bq[2 * H:]], 0)            # (2, H)

    # ---- ff: fold gamma/beta ----
    wf = ff_w * ff_ln_g[None, :]
    bf_ = ff_b + ff_w @ ff_ln_b
    csff = wf.sum(1)[None, :]                             # (1, H)

    eb4 = np.zeros((128, BPC * 64), np.float32)
    eb4[:, 0::64] = 1.0 / H

    def sbpack(w):
        # (H_in, N) -> [128, G*N]: partition p, col g*N+n = w[128g+p, n]
        n = w.shape[1]
        return np.ascontiguousarray(
            w.reshape(G, 128, n).transpose(1, 0, 2).reshape(128, G * n))

    common = {
        "wpw": _bf16(np.stack([sbpack(pw_w[i].T) for i in range(NC)])),
        "dwdiag": _bf16(dwdiag),
        "wj": wj_pack,
        "pwb": vecpack(eff_pwb),
        "wqkv": _bf16(sbpack(wq.T)),
        "csqkv": np.ascontiguousarray(csqkv),
        "csv": np.ascontiguousarray(csv),
        "wout": _bf16(sbpack(out_w.T)),
        "bout": vecpack(out_b),
        "wff": _bf16(sbpack(wf.T)),
        "csff": np.ascontiguousarray(csff),
        "bff": vecpack(bf_),
        "eb4": _bf16(eb4),
        "ones1": np.ones((1, 128), np.float32),
        "ones1f": np.ones((1, 64), np.float32),
        "voinit": _bf16(np.tile(
            np.concatenate([np.zeros((1, NH, D), np.float32),
                            np.ones((1, NH, 1), np.float32)],
                           2).reshape(1, NH * (D + 1)), (128, 1))),
        "nrowi": np.ascontiguousarray(np.concatenate(
            [np.zeros((1, LP), np.float32),
             np.pad(np.ones((1, L), np.float32),
                    ((0, 0), (4, LP - L - 4)))], 0)),
    }
    mask_all_ones = bool((np.asarray(mask) > 0).all())
    mbs = []
    for c in range(NCORES):
        mbc = np.zeros((128, BPC * G), np.float32)
        for b in range(BPC):
            mrow = np.asarray(mask[c * BPC + b])
            for ch in range(G):
                mbc[0:KC, b * G + ch] = np.where(
                    mrow[KC * ch:KC * (ch + 1)] > 0, 0.0, -1e9)
        mbs.append(mbc)
    # (B, H, L) -> per-batch [128, G*L] with group g at cols [g*L, (g+1)*L)
    xp8 = np.ascontiguousarray(
        xpe.reshape(B, G, 128, L).transpose(0, 2, 1, 3).reshape(B, 128, G * L))
    x_shards = [_bf16(xp8[c * BPC:(c + 1) * BPC]) for c in range(NCORES)]
    return common, x_shards, mbs, mask_all_ones


def get_program(mask_all_ones=True, repeat=1):
    key = ("prog", mask_all_ones, repeat)
    if key not in _cache:
        _cache[key] = _build_program(mask_all_ones, repeat)
    return _cache[key]


def make_in_maps(inputs):
    common, x_shards, mbs, mask_all_ones = _prep_inputs(inputs)
    in_maps = []
    for c in range(NCORES):
        m = dict(common)
        m["xT"] = x_shards[c]
        m["mb"] = mbs[c]
        in_maps.append(m)
    return in_maps, mask_all_ones


def gather_output(results):
    outs = []
    for c in range(NCORES):
        r = np.asarray(results[c]["outT"], np.float32)   # [BPC, 128, G*L]
        r = r.reshape(BPC, 128, G, L).transpose(0, 3, 2, 1).reshape(BPC, L, H)
        outs.append(r)
    return np.ascontiguousarray(np.concatenate(outs, axis=0), dtype=np.float32)


def kernel(**inputs):
    from concourse.bass_utils import run_bass_kernel_spmd

    in_maps, mask_all_ones = make_in_maps(inputs)
    nc = get_program(mask_all_ones)
    res = run_bass_kernel_spmd(nc, in_maps, list(range(NCORES)))
    return gather_output(res.results)


# revision 27
# speedup vs baseline: 1.2944x; 1.2944x over previous
"""Trainium2 Bass kernel for nn_EncoderBlock (conformer-style encoder block).

Model: x+posenc -> 4x [LN -> depthwise conv(K=7) -> pointwise HxH -> ReLU -> +res]
       -> [LN -> MHA(8 heads, d=64) -> +res] -> [LN -> HxH -> ReLU -> +res]
B=32, L=400, H=512.

Sharding: pure data-parallel over batch. 8 cores x 4 batches, weights
replicated, no collectives.

bf16 datapath: activations, weights, and norm tiles are bf16 (DVE
tensor_tensor in 2x mode, copies 4x; matmuls full-rate with fp32 PSUM).
LN gamma/beta fold into downstream weights host-side; the per-token LN shift
(-m*s) rides into every matmul as a K=1/K=2 accumulation against a
[shift-row; ones-row] pair, so the materialized norm is just y = x*s — one
wide tensor_tensor per batch (all 4 channel groups live in one padded
[128, 4*LP] tile). Depthwise-conv taps run on the PE as 7 accumulating
matmuls per group against host-built diagonal weight matrices; the LN-shift
contribution enters the pointwise PSUM via a K=7 matmul against 7 shifted
copies of the shift row (built by tiny SWDGE sbuf->sbuf DMAs on the idle
GPSIMD queue). PSUM->SBUF moves ride ACT (ReLU/bias fused) except in the
ACT-heavy attention phase, where the softmax normalize runs on DVE reading
the AV PSUM directly. Weights load as one batched multi-dim-AP DMA per
matrix, queued behind the activation loads so compute starts immediately.
Softmax uses KC=100 key chunks (paired exp needs no memset) and gets
sum(exp) free from a ones column appended to V^T.
"""

import math
import sys

import numpy as np

for _p in ("/opt/trn_rl_repo",):
    if _p not in sys.path:
        sys.path.append(_p)

B, L, H, NH, K, NC = 32, 400, 512, 8, 7, 4
D = H // NH          # 64
NCORES = 8
BPC = B // NCORES    # batches per core
G = H // 128         # 4 feature groups
KC = 100             # key chunk of L=400 (4 chunks)
SR = 97              # stats tile rows; batch b's row = 32*b
PAD = K // 2         # 3
LP = L + 8           # padded slot width: y data at col 4
EPS = 1e-5
H3 = 3 * H

_cache = {}


def _bf16(a):
    import ml_dtypes
    return np.asarray(a, np.float32).astype(ml_dtypes.bfloat16)


def _pos_encoding():
    pos = np.arange(L, dtype=np.float64)[:, None]
    div = np.exp(np.arange(0, H, 2, dtype=np.float64) * (-math.log(10000.0) / H))
    pe = np.zeros((L, H), np.float64)
    pe[:, 0::2] = np.sin(pos * div)
    pe[:, 1::2] = np.cos(pos * div)
    return pe.astype(np.float32)


def _install_drain_patch(tile_mod):
    """This walrus build rejects >2 sem waits on one control instruction
    ("Too many sync wait commands"). TileContext's closing drain carries the
    whole global clock; spread it over 1-wait nops instead."""
    from concourse.vector_clock import ScopedClock, VectorClock

    def _patched_drain_and_barrier(self, tick_clock, wait_clock):
        gc = tick_clock.global_clock
        n = len(gc)
        for i in range(n):
            t = gc[i]
            if t > 0:
                nop = self.nc.sync.nop()
                vec = [0] * n
                vec[i] = t
                wait_clock.add_sem_waits(nop.ins, ScopedClock({None: VectorClock(vec)}))
        self.nc.sync.drain()
        self.nc.all_engine_barrier()
        assert self.sems is not None
        popped = self.nc._tile_sem_poison_stack.pop()
        assert popped is self._sem_poison
        self.nc.clear_and_free_semaphores(list(self.sems.allocated().values()))
        self.nc.all_engine_barrier()

    tile_mod.TileContext._drain_and_barrier = _patched_drain_and_barrier


def _split_excess_waits(nc, mybir, limit=1):
    """This walrus build caps semaphore waits per instruction. Move excess
    waits onto same-engine InstNoOp carriers placed just before the
    instruction (engines execute their stream in order, so ordering is
    preserved)."""
    import bass_rust
    for f in nc.m.functions:
        for blk in f.blocks:
            out = []
            changed = False
            for inst in blk.instructions:
                si = inst.sync_info
                waits = list(si.on_wait) if si is not None and si.on_wait else []
                if len(waits) > limit:
                    head, keep = waits[:-limit], waits[-limit:]
                    for j, w in enumerate(head):
                        nop = bass_rust.InstNoOp(
                            name=f"{inst.name}_w{j}", ins=[], outs=[])
                        nop.engine = inst.engine
                        nop.sync_info = mybir.SyncInfo(on_wait=[w],
                                                       on_update=[])
                        nc.register_instruction(nop)
                        out.append(nop)
                    si.on_wait = keep
                    changed = True
                out.append(inst)
            if changed:
                blk.instructions[:] = out


def _build_program(mask_all_ones, repeat=1):
    import concourse.bass as bass
    import concourse.mybir as mybir
    import concourse.tile as tile

    _install_drain_patch(tile)

    f32 = mybir.dt.float32
    f32r = mybir.dt.float32r
    bf16 = mybir.dt.bfloat16
    AF = mybir.ActivationFunctionType
    OP = mybir.AluOpType

    nc = bass.Bass("TRN2", target_bir_lowering=False, debug=False,
                   num_devices=NCORES)

    # ---------------- DRAM parameters (per core) ----------------
    xT = nc.dram_tensor("xT", [BPC, 128, G * L], bf16, kind="ExternalInput").ap()
    wpw = nc.dram_tensor("wpw", [NC, 128, G * H], bf16, kind="ExternalInput").ap()
    dwdiag = nc.dram_tensor("dwdiag", [NC, 128, K * G * 128], bf16,
                            kind="ExternalInput").ap()
    wj = nc.dram_tensor("wj", [K, NC * H], f32r, kind="ExternalInput").ap()
    pwb = nc.dram_tensor("pwb", [128, NC * G], f32, kind="ExternalInput").ap()
    wqkv = nc.dram_tensor("wqkv", [128, G * H3], bf16, kind="ExternalInput").ap()
    csqkv = nc.dram_tensor("csqkv", [2, 2 * H], f32r, kind="ExternalInput").ap()
    csv = nc.dram_tensor("csv", [2, H], f32r, kind="ExternalInput").ap()
    wout = nc.dram_tensor("wout", [128, G * H], bf16, kind="ExternalInput").ap()
    bout = nc.dram_tensor("bout", [128, G], f32, kind="ExternalInput").ap()
    wff = nc.dram_tensor("wff", [128, G * H], bf16, kind="ExternalInput").ap()
    csff = nc.dram_tensor("csff", [1, H], f32r, kind="ExternalInput").ap()
    bff = nc.dram_tensor("bff", [128, G], f32, kind="ExternalInput").ap()
    eb4 = nc.dram_tensor("eb4", [128, BPC * 64], bf16, kind="ExternalInput").ap()
    ones1 = nc.dram_tensor("ones1", [1, 128], f32r, kind="ExternalInput").ap()
    ones1f = nc.dram_tensor("ones1f", [1, 64], f32r, kind="ExternalInput").ap()
    nrowi = nc.dram_tensor("nrowi", [2, LP], f32r, kind="ExternalInput").ap()
    voinit = nc.dram_tensor("voinit", [128, NH * (D + 1)], bf16,
                            kind="ExternalInput").ap()
    mb = nc.dram_tensor("mb", [128, BPC * G], f32, kind="ExternalInput").ap()
    outT = nc.dram_tensor("outT", [BPC, 128, G * L], bf16,
                          kind="ExternalOutput").ap()

    with tile.TileContext(nc) as tc:
        with (
            tc.tile_pool(name="const", bufs=1) as constp,
            tc.tile_pool(name="xp", bufs=1) as xp,
            tc.tile_pool(name="wp", bufs=1) as wp,
            tc.tile_pool(name="scr", bufs=2) as scr,
            tc.tile_pool(name="attp", bufs=1) as attp,
            tc.tile_pool(name="exptp", bufs=2) as exptp,
            tc.tile_pool(name="ps", bufs=2, space="PSUM") as ps,
        ):
            c_zero = constp.tile([128, 1], f32, tag="c_zero")
            nc.vector.memset(c_zero[:], 0.0)
            c_eps = constp.tile([128, 1], f32, tag="c_eps")
            nc.vector.memset(c_eps[:], EPS)
            nc.const_aps.aps[(f32, 0.0)] = c_zero[:]
            nc.const_aps.aps[(f32, EPS)] = c_eps[:]

            def const_tile(shape, src, nm, dt=f32):
                t = constp.tile(shape, dt, tag=nm, name=nm)
                nc.sync.dma_start(t[:], src)
                return t

            # y tiles: one [128, G*LP] padded tile per batch parity; data for
            # group g at cols [g*LP+4, g*LP+4+L). Margins zeroed once.
            y4 = [constp.tile([128, G * LP], bf16, tag=f"y4_{p}",
                              name=f"y4_{p}") for p in range(2)]
            y4o = [constp.tile([128, G * LP], bf16, tag=f"y4o_{p}",
                               name=f"y4o_{p}") for p in range(2)]
            for p in range(2):
                nc.vector.memset(y4[p][:], 0.0)
                nc.vector.memset(y4o[p][:], 0.0)
            # per-batch shift/ones row pairs: row0 = -m*s (data at col 4),
            # row1 = ones over the data region
            nrow2 = [constp.tile([2, LP], f32r, tag=f"nrow2_{b}",
                                 name=f"nrow2_{b}")
                     for b in range(BPC)]
            for b in range(BPC):
                nc.sync.dma_start(nrow2[b][:], nrowi[:])
            # persistent v^T tiles with ones column pre-set
            vo = [[constp.tile([128, NH * (D + 1)], bf16,
                               tag=f"vo_{c}_{p}", name=f"vo_{c}_{p}")
                   for p in range(2)] for c in range(G)]
            for c in range(G):
                for p in range(2):
                    nc.sync.dma_start(vo[c][p][:], voinit[:])

            for _rep in range(repeat):
                # -------- activation loads first, then consts, then weights
                # in use order (single HWDGE queue drains in issue order) ----
                xt = []      # per-batch [128, G*L] residual tiles
                for b in range(BPC):
                    t = xp.tile([128, G * L], bf16, tag=f"x_{b}", name=f"x_{b}")
                    nc.sync.dma_start(t[:], xT[b])
                    xt.append(t)

                c_eb4 = const_tile([128, BPC * 64], eb4[:], 'eb4', bf16)
                c_ones1 = const_tile([1, 128], ones1[:], 'ones1', f32r)
                c_ones1f = const_tile([1, 64], ones1f[:], 'ones1f', f32r)
                c_wj = const_tile([K, NC * H], wj[:], 'wj', f32r)
                c_pwb = const_tile([128, NC * G], pwb[:], 'pwb')

                w_pw_all, w_dg = [], []
                for i in range(NC):
                    t = wp.tile([128, G * H], bf16, tag=f"pw_{i}", name=f"pw_{i}")
                    nc.sync.dma_start(t[:], wpw[i])
                    w_pw_all.append(t)
                    t = wp.tile([128, K * G * 128], bf16, tag=f"dg_{i}",
                                name=f"dg_{i}")
                    nc.sync.dma_start(t[:], dwdiag[i])
                    w_dg.append(t)

                w_qkv = wp.tile([128, G * H3], bf16, tag="qkv", name="qkv")
                nc.sync.dma_start(w_qkv[:], wqkv[:])
                c_csqkv = const_tile([2, 2 * H], csqkv[:], 'csqkv', f32r)
                c_csv = const_tile([2, H], csv[:], 'csv', f32r)
                w_out = wp.tile([128, G * H], bf16, tag="out", name="out")
                nc.sync.dma_start(w_out[:], wout[:])
                c_bout = const_tile([128, G], bout[:], 'bout')
                w_ff = wp.tile([128, G * H], bf16, tag="ff", name="ff")
                nc.sync.dma_start(w_ff[:], wff[:])
                c_csff = const_tile([1, H], csff[:], 'csff', f32r)
                c_bff = const_tile([128, G], bff[:], 'bff')
                c_mb = None if mask_all_ones else const_tile(
                    [128, BPC * G], mb[:], 'mb')

                def xsl(b, g):
                    return xt[b][:, L * g:L * (g + 1)]

                def ysl(p, g, j):
                    """y view of group g shifted for tap j (x-hat[l+j-3]),
                    always 4-byte aligned: odd j reads y4 (data at col 4),
                    even j reads the one-right-shifted copy y4o (col 5)."""
                    if j % 2 == 1:
                        return y4[p][:, LP * g + j + 1:LP * g + j + 1 + L]
                    return y4o[p][:, LP * g + j + 2:LP * g + j + 2 + L]

                # ---------- layer norm ----------
                def layer_norm(tag, consume, sq_dve=False):
                    """LN over H, batch-pipelined: each batch gets its own
                    [2, L] stats PSUM (row0 = E[x], row1 = E[x^2]) so batch
                    b's consume starts as soon as its own 8 stats matmuls
                    and short fp32 chain finish. ACT Exp writes the bf16 s
                    row directly; the shift row -m*s lands straight in
                    nrow2[b]."""
                    with tc.tile_pool(name=f"pst_{tag}", bufs=1,
                                      space="PSUM") as pst:
                        for b in range(BPC):
                            p = b % 2
                            ebT = c_eb4[:, 64 * b:64 * (b + 1)]
                            ex1 = pst.tile([64, L], f32, tag="st1",
                                           name=f"ex1_{tag}_{b}")
                            ex2 = pst.tile([64, L], f32, tag="st2",
                                           name=f"ex2_{tag}_{b}")
                            for g in range(G):
                                nc.tensor.matmul(ex1[:], ebT, xsl(b, g),
                                                 start=(g == 0),
                                                 stop=(g == G - 1))
                            sq = scr.tile([128, G * L], bf16, tag="sq",
                                          name=f"sq_{tag}_{b}")
                            if sq_dve:
                                nc.vector.tensor_tensor(sq[:], xt[b][:],
                                                        xt[b][:], op=OP.mult)
                            else:
                                nc.scalar.activation(sq[:], xt[b][:], AF.Square)
                            for g in range(G):
                                nc.tensor.matmul(ex2[:], ebT,
                                                 sq[:, L * g:L * (g + 1)],
                                                 start=(g == 0),
                                                 stop=(g == G - 1))
                            # (DVE may read only one PSUM operand per op)
                            m_sb = scr.tile([1, L], f32, tag="ln_m", bufs=3,
                                            name=f"m_{tag}_{b}")
                            nc.vector.tensor_copy(m_sb[:], ex1[0:1, :])
                            msq = scr.tile([1, L], f32, tag="ln_tmp", bufs=3,
                                           name=f"msq_{tag}_{b}")
                            nc.vector.tensor_tensor(msq[:], m_sb[:],
                                                    ex1[0:1, :], op=OP.mult)
                            var = scr.tile([1, L], f32, tag="ln_tmp", bufs=3,
                                           name=f"var_{tag}_{b}")
                            nc.vector.scalar_tensor_tensor(
                                var[:], msq[:], -1.0, ex2[0:1, :],
                                op0=OP.mult, op1=OP.add)
                            lnv = scr.tile([1, L], f32, tag="ln_tmp", bufs=3,
                                           name=f"lnv_{tag}_{b}")
                            nc.scalar.activation(lnv[:], var[:], AF.Ln, bias=EPS)
                            s1 = scr.tile([1, L], f32r, tag="ln_s1", bufs=4,
                                          name=f"s1_{tag}_{b}")
                            nc.scalar.activation(s1[:], lnv[:], AF.Exp,
                                                 scale=-0.5)
                            nc.vector.scalar_tensor_tensor(
                                nrow2[b][0:1, 4:4 + L], m_sb[:], -1.0, s1[:],
                                op0=OP.mult, op1=OP.mult)
                            sbp = ps.tile([128, L], f32, tag="mm",
                                          name=f"sb_{tag}_{b}")
                            nc.tensor.matmul(sbp[:], c_ones1[:], s1[:],
                                             start=True, stop=True)
                            sb = scr.tile([128, L], bf16, tag="sb", bufs=2,
                                          name=f"sbs_{tag}_{b}")
                            nc.scalar.activation(sb[:], sbp[:], AF.Copy)
                            for g in range(G):
                                nc.vector.tensor_tensor(
                                    y4[p][:, LP * g + 4:LP * g + 4 + L],
                                    xsl(b, g), sb[:], op=OP.mult)
                            consume(b, p)

                # ================= conv blocks =================
                for i in range(NC):
                    def conv_consume(b, p, i=i):
                        # one-right-shifted copy of y for the even taps
                        nc.sync.dma_start(
                            y4o[p][:].rearrange("q (g l) -> q g l", l=LP)
                            [:, :, 5:5 + L],
                            y4[p][:].rearrange("q (g l) -> q g l", l=LP)
                            [:, :, 4:4 + L])
                        # shifted copies of the -m*s row for the K=7 shift-
                        # correction matmul (reads padded source)
                        nt = scr.tile([K, L], f32r, tag="nshift", bufs=2,
                                      name=f"nt_{i}_{b}")
                        for j in range(K):
                            nc.sync.dma_start(nt[j:j + 1, :],
                                              nrow2[b][0:1, j + 1:j + 1 + L])
                        acc = scr.tile([128, G * L], bf16, tag="acc", bufs=2,
                                       name=f"acc_{i}_{b}")
                        with tc.tile_pool(name=f"ptap_{i}_{b}", bufs=2,
                                          space="PSUM") as ptap, \
                             tc.tile_pool(name=f"pmc_{i}_{b}", bufs=2,
                                          space="PSUM") as pmc:
                            for g in range(G):
                                pt = ptap.tile([128, L], f32, tag="tap",
                                               name=f"pt_{i}_{b}_{g}")
                                jorder = (1, 3, 5, 0, 2, 4, 6)
                                for ji, j in enumerate(jorder):
                                    nc.tensor.matmul(
                                        pt[:],
                                        w_dg[i][:, (j * G + g) * 128:
                                                (j * G + g + 1) * 128],
                                        ysl(p, g, j),
                                        start=(ji == 0), stop=(ji == K - 1))
                                nc.scalar.activation(
                                    acc[:, L * g:L * (g + 1)], pt[:], AF.Copy)
                            cvall = scr.tile([128, G * L], bf16, tag="cv",
                                             name=f"cv_{i}_{b}")
                            for o in range(G):
                                pm = pmc.tile([128, L], f32, tag="pm",
                                              name=f"pm_{i}_{b}_{o}")
                                for g in range(G):
                                    nc.tensor.matmul(
                                        pm[:],
                                        w_pw_all[i][:, H * g + 128 * o:
                                                    H * g + 128 * (o + 1)],
                                        acc[:, L * g:L * (g + 1)],
                                        start=(g == 0), stop=False)
                                nc.tensor.matmul(
                                    pm[:],
                                    c_wj[:, i * H + 128 * o:i * H + 128 * (o + 1)],
                                    nt[:], start=False, stop=True)
                                nc.scalar.activation(
                                    cvall[:, L * o:L * (o + 1)], pm[:], AF.Relu,
                                    bias=c_pwb[:, i * G + o:i * G + o + 1])
                            nc.vector.tensor_tensor(xt[b][:], xt[b][:],
                                                    cvall[:], op=OP.add)

                    layer_norm(f"c{i}", conv_consume)

                # ================= attention =================
                def att_consume(b, p):
                    qsb, ksb = [], []
                    for m in range(2 * G):
                        pm = ps.tile([128, L], f32, tag="mm", name=f"pqk_{b}_{m}")
                        for g in range(G):
                            nc.tensor.matmul(
                                pm[:],
                                w_qkv[:, H3 * g + 128 * m:H3 * g + 128 * (m + 1)],
                                y4[p][:, LP * g + 4:LP * g + 4 + L],
                                start=(g == 0), stop=False)
                        nc.tensor.matmul(
                            pm[:], c_csqkv[:, 128 * m:128 * (m + 1)],
                            nrow2[b][0:2, 4:4 + L], start=False, stop=True)
                        t = attp.tile([128, L], bf16, tag=f"qk_{m}",
                                      bufs=2, name=f"qk_{b}_{m}")
                        nc.scalar.activation(t[:], pm[:], AF.Copy)
                        (qsb if m < G else ksb).append(t)
                    # v pre-transposed with bias+shift via K=2 row pair;
                    # ones column pre-set gives sum(exp) free in AV
                    for c in range(G):
                        pv = ps.tile([128, H], f32, tag="mm", name=f"pv_{b}_{c}")
                        for g in range(G):
                            nc.tensor.matmul(
                                pv[0:KC, :],
                                y4[p][:, LP * g + 4 + KC * c:
                                      LP * g + 4 + KC * (c + 1)],
                                w_qkv[:, H3 * g + 2 * H:H3 * g + 3 * H],
                                start=(g == 0), stop=False)
                        nc.tensor.matmul(
                            pv[0:KC, :],
                            nrow2[b][0:2, 4 + KC * c:4 + KC * (c + 1)],
                            c_csv[:], start=False, stop=True)
                        vv = vo[c][p][:].rearrange("p (h d) -> p h d", d=D + 1)
                        pvv = pv[:, 0:H].rearrange("p (h d) -> p h d", d=D)
                        nc.scalar.activation(vv[0:KC, :, 0:D], pvv[0:KC, :, :],
                                             AF.Copy)
                    att = []
                    for g in range(G):
                        att.append(attp.tile([128, L], bf16, tag=f"ao_{g}",
                                             bufs=2, name=f"ao_{b}_{g}"))
                    with tc.tile_pool(name=f"psatt_{b}", bufs=2,
                                      space="PSUM") as psatt:
                        for h in range(NH):
                            g2, ho = h // 2, (h % 2) * 64
                            # scores^T [k, q], one 1-bank psum per key chunk
                            ex = exptp.tile([128, G * L], bf16, tag="expT",
                                            bufs=3, name=f"ex_{b}_{h}")
                            for c in range(G):
                                sc = psatt.tile([128, 512], f32, tag="sc",
                                                name=f"sc_{b}_{h}_{c}")
                                nc.tensor.matmul(
                                    sc[0:KC, 0:L],
                                    ksb[g2][ho:ho + 64, KC * c:KC * (c + 1)],
                                    qsb[g2][ho:ho + 64, :],
                                    start=True, stop=True)
                                if mask_all_ones:
                                    nc.scalar.activation(
                                        ex[0:KC, L * c:L * c + L],
                                        sc[0:KC, 0:L], AF.Exp)
                                else:
                                    nc.scalar.activation(
                                        ex[0:KC, L * c:L * c + L],
                                        sc[0:KC, 0:L], AF.Exp,
                                        bias=c_mb[0:KC,
                                                  b * G + c:b * G + c + 1])
                            # AV with ones row -> [65, L]; row 64 = sum(exp)
                            av = psatt.tile([D + 1, L], f32, tag="av",
                                            name=f"av_{b}_{h}")
                            for c in range(G):
                                nc.tensor.matmul(
                                    av[:],
                                    vo[c][p][0:KC,
                                             (D + 1) * h:(D + 1) * (h + 1)],
                                    ex[0:KC, L * c:L * c + L],
                                    start=(c == 0), stop=(c == G - 1))
                            inv = scr.tile([1, L], f32r, tag="inv",
                                           name=f"inv_{b}_{h}")
                            with nc.allow_low_precision(reason="f32r feed"):
                                nc.vector.reciprocal(inv[:], av[D:D + 1, :])
                            # inv broadcast rides the sc tag rotation (slot
                            # freed once this head's exps are read)
                            ib = psatt.tile([64, 512], f32, tag="sc",
                                            name=f"ib_{b}_{h}")
                            nc.tensor.matmul(ib[:, 0:L], c_ones1f[:], inv[:],
                                             start=True, stop=True)
                            ibs = scr.tile([64, L], bf16, tag="ibs",
                                           name=f"ibs_{b}_{h}")
                            nc.vector.tensor_copy(ibs[:], ib[:, 0:L])
                            nc.vector.tensor_tensor(att[g2][ho:ho + 64, :],
                                                    av[0:D, :], ibs[:],
                                                    op=OP.mult)
                    # output projection + residual for this batch
                    aoall = scr.tile([128, G * L], bf16, tag="cv",
                                     name=f"aa_{b}")
                    for o in range(G):
                        pm = ps.tile([128, L], f32, tag="mm", name=f"po_{b}_{o}")
                        for g in range(G):
                            nc.tensor.matmul(
                                pm[:],
                                w_out[:, H * g + 128 * o:H * g + 128 * (o + 1)],
                                att[g][:], start=(g == 0), stop=(g == G - 1))
                        nc.vector.tensor_scalar_add(
                            aoall[:, L * o:L * (o + 1)], pm[:],
                            c_bout[:, o:o + 1])
                    nc.vector.tensor_tensor(xt[b][:], xt[b][:], aoall[:],
                                            op=OP.add)

                layer_norm("a", att_consume, sq_dve=True)

                # ================= feed-forward =================
                def ff_consume(b, p):
                    foall = scr.tile([128, G * L], bf16, tag="cv",
                                     name=f"fo_{b}")
                    for o in range(G):
                        pm = ps.tile([128, L], f32, tag="mm", name=f"pff_{b}_{o}")
                        for g in range(G):
                            nc.tensor.matmul(
                                pm[:],
                                w_ff[:, H * g + 128 * o:H * g + 128 * (o + 1)],
                                y4[p][:, LP * g + 4:LP * g + 4 + L],
                                start=(g == 0), stop=False)
                        nc.tensor.matmul(
                            pm[:], c_csff[:, 128 * o:128 * (o + 1)],
                            nrow2[b][0:1, 4:4 + L], start=False, stop=True)
                        nc.scalar.activation(foall[:, L * o:L * (o + 1)], pm[:],
                                             AF.Relu, bias=c_bff[:, o:o + 1])
                    # final residual add (bf16 store; host converts to f32)
                    xo = scr.tile([128, G * L], bf16, tag="xo", bufs=2,
                                  name=f"xo_{b}")
                    nc.vector.tensor_tensor(xo[:], xt[b][:], foall[:], op=OP.add)
                    nc.sync.dma_start(outT[b], xo[:])

                layer_norm("f", ff_consume, sq_dve=True)
    _split_excess_waits(nc, mybir, limit=1)
    return nc


def _prep_inputs(inputs):
    """Host-side packing shared by all cores."""
    x = np.asarray(inputs["x"], np.float32)
    mask = np.asarray(inputs["mask"])
    pe = _pos_encoding()
    xpe = np.ascontiguousarray((x + pe[None]).transpose(0, 2, 1))  # (B, H, L)

    conv_ln_g = np.asarray(inputs["conv_ln_g"], np.float32)
    conv_ln_b = np.asarray(inputs["conv_ln_b"], np.float32)
    dw_w = np.asarray(inputs["dw_w"], np.float32)[:, :, 0, :]  # (NC, H, K)
    dw_b = np.asarray(inputs["dw_b"], np.float32)
    pw_w = np.asarray(inputs["pw_w"], np.float32)        # (NC, H_out, H_in)
    pw_b = np.asarray(inputs["pw_b"], np.float32)
    att_ln_g = np.asarray(inputs["att_ln_g"], np.float32)
    att_ln_b = np.asarray(inputs["att_ln_b"], np.float32)
    qkv_w = np.asarray(inputs["qkv_w"], np.float32)      # (3H, H)
    qkv_b = np.asarray(inputs["qkv_b"], np.float32)
    out_w = np.asarray(inputs["out_w"], np.float32)
    out_b = np.asarray(inputs["out_b"], np.float32)
    ff_ln_g = np.asarray(inputs["ff_ln_g"], np.float32)
    ff_ln_b = np.asarray(inputs["ff_ln_b"], np.float32)
    ff_w = np.asarray(inputs["ff_w"], np.float32)
    ff_b = np.asarray(inputs["ff_b"], np.float32)

    def vecpack(v):  # (..., H) -> [128, n*G], col n*G-index = (outer, g)
        v = np.asarray(v, np.float32).reshape(-1, G, 128)
        return np.ascontiguousarray(v.transpose(2, 0, 1).reshape(128, -1))

    # ---- conv blocks: fold LN gamma into dw weights, beta into pw bias ----
    dwp = dw_w * conv_ln_g[:, :, None]                    # (NC, H, K) folded
    const_c = conv_ln_b * dw_w.sum(-1)                    # beta * sum_j dw
    eff_pwb = pw_b + np.einsum("ioc,ic->io", pw_w, dw_b + const_c)
    # K=7 shift-correction weights: WJ[i][j, o] = sum_c pw[o,c]*dwp[c,j]
    wj_all = np.einsum("ioc,icj->ijo", pw_w, dwp)         # (NC, K, H)
    wj_pack = np.ascontiguousarray(
        wj_all.transpose(1, 0, 2).reshape(K, NC * H))
    # diag matrices for the PE taps, pre-packed to the SBUF tile layout:
    # [i, p, (j*G+g)*128 + c] = dwp[i, 128g+p, j] if c == p else 0
    dwdiag = np.zeros((NC, 128, K * G * 128), np.float32)
    for i in range(NC):
        for j in range(K):
            for g in range(G):
                blk = (j * G + g) * 128
                dwdiag[i, np.arange(128), blk + np.arange(128)] = \
                    dwp[i, 128 * g:128 * (g + 1), j]

    # ---- attention: fold gamma into W rows, beta into bias; scale q ----
    scale = 1.0 / math.sqrt(D)
    wq = qkv_w * att_ln_g[None, :]
    bq = qkv_b + qkv_w @ att_ln_b
    wq[:H] *= scale
    bq[:H] *= scale
    cs = wq.sum(1)                                        # (3H,)
    csqkv = np.stack([cs[:2 * H], bq[:2 * H]], 0)          # (2, 2H)
    csv = np.stack([cs[2 * H:], bq[2 * H:]], 0)            # (2, H)

    # ---- ff: fold gamma/beta ----
    wf = ff_w * ff_ln_g[None, :]
    bf_ = ff_b + ff_w @ ff_ln_b
    csff = wf.sum(1)[None, :]                             # (1, H)

    eb4 = np.zeros((128, BPC * 64), np.float32)
    eb4[:, 0::64] = 1.0 / H

    def sbpack(w):
        # (H_in, N) -> [128, G*N]: partition p, col g*N+n = w[128g+p, n]
        n = w.shape[1]
        return np.ascontiguousarray(
            w.reshape(G, 128, n).transpose(1, 0, 2).reshape(128, G * n))

    common = {
        "wpw": _bf16(np.stack([sbpack(pw_w[i].T) for i in range(NC)])),
        "dwdiag": _bf16(dwdiag),
        "wj": wj_pack,
        "pwb": vecpack(eff_pwb),
        "wqkv": _bf16(sbpack(wq.T)),
        "csqkv": np.ascontiguousarray(csqkv),
        "csv": np.ascontiguousarray(csv),
        "wout": _bf16(sbpack(out_w.T)),
        "bout": vecpack(out_b),
        "wff": _bf16(sbpack(wf.T)),
        "csff": np.ascontiguousarray(csff),
        "bff": vecpack(bf_),
        "eb4": _bf16(eb4),
        "ones1": np.ones((1, 128), np.float32),
        "ones1f": np.ones((1, 64), np.float32),
        "voinit": _bf16(np.tile(
            np.concatenate([np.zeros((1, NH, D), np.float32),
                            np.ones((1, NH, 1), np.float32)],
                           2).reshape(1, NH * (D + 1)), (128, 1))),
        "nrowi": np.ascontiguousarray(np.concatenate(
            [np.zeros((1, LP), np.float32),
             np.pad(np.ones((1, L), np.float32),
                    ((0, 0), (4, LP - L - 4)))], 0)),
    }
    mask_all_ones = bool((np.asarray(mask) > 0).all())
    mbs = []
    for c in range(NCORES):
        mbc = np.zeros((128, BPC * G), np.float32)
        for b in range(BPC):
            mrow = np.asarray(mask[c * BPC + b])
            for ch in range(G):
                mbc[0:KC, b * G + ch] = np.where(
                    mrow[KC * ch:KC * (ch + 1)] > 0, 0.0, -1e9)
        mbs.append(mbc)
    # (B, H, L) -> per-batch [128, G*L] with group g at cols [g*L, (g+1)*L)
    xp8 = np.ascontiguousarray(
        xpe.reshape(B, G, 128, L).transpose(0, 2, 1, 3).reshape(B, 128, G * L))
    x_shards = [_bf16(xp8[c * BPC:(c + 1) * BPC]) for c in range(NCORES)]
    return common, x_shards, mbs, mask_all_ones


def get_program(mask_all_ones=True, repeat=1):
    key = ("prog", mask_all_ones, repeat)
    if key not in _cache:
        _cache[key] = _build_program(mask_all_ones, repeat)
    return _cache[key]


def make_in_maps(inputs):
    common, x_shards, mbs, mask_all_ones = _prep_inputs(inputs)
    in_maps = []
    for c in range(NCORES):
        m = dict(common)
        m["xT"] = x_shards[c]
        m["mb"] = mbs[c]
        in_maps.append(m)
    return in_maps, mask_all_ones


def gather_output(results):
    outs = []
    for c in range(NCORES):
        r = np.asarray(results[c]["outT"], np.float32)   # [BPC, 128, G*L]
        r = r.reshape(BPC, 128, G, L).transpose(0, 3, 2, 1).reshape(BPC, L, H)
        outs.append(r)
    return np.ascontiguousarray(np.concatenate(outs, axis=0), dtype=np.float32)


def kernel(**inputs):
    from concourse.bass_utils import run_bass_kernel_spmd

    in_maps, mask_all_ones = make_in_maps(inputs)
    nc = get_program(mask_all_ones)
    res = run_bass_kernel_spmd(nc, in_maps, list(range(NCORES)))
    return gather_output(res.results)


# revision 28
# speedup vs baseline: 2.5964x; 2.0058x over previous
"""Trainium2 Bass kernel for nn_EncoderBlock (conformer-style encoder block).

Model: x+posenc -> 4x [LN -> depthwise conv(K=7) -> pointwise HxH -> ReLU -> +res]
       -> [LN -> MHA(8 heads, d=64) -> +res] -> [LN -> HxH -> ReLU -> +res]
B=32, L=400, H=512.

Sharding: pure data-parallel over batch. 8 cores x 4 batches, weights
replicated, no collectives.

bf16 datapath: activations, weights, and norm tiles are bf16 (DVE
tensor_tensor in 2x mode, copies 4x; matmuls full-rate with fp32 PSUM).
LN gamma/beta fold into downstream weights host-side; the per-token LN shift
(-m*s) rides into every matmul as a K=1/K=2 accumulation against a
[shift-row; ones-row] pair, so the materialized norm is just y = x*s — one
wide tensor_tensor per batch (all 4 channel groups live in one padded
[128, 4*LP] tile). Depthwise-conv taps run on the PE as 7 accumulating
matmuls per group against host-built diagonal weight matrices; the LN-shift
contribution enters the pointwise PSUM via a K=7 matmul against 7 shifted
copies of the shift row (built by tiny SWDGE sbuf->sbuf DMAs on the idle
GPSIMD queue). PSUM->SBUF moves ride ACT (ReLU/bias fused) except in the
ACT-heavy attention phase, where the softmax normalize runs on DVE reading
the AV PSUM directly. Weights load as one batched multi-dim-AP DMA per
matrix, queued behind the activation loads so compute starts immediately.
Softmax uses KC=100 key chunks (paired exp needs no memset) and gets
sum(exp) free from a ones column appended to V^T.
"""

import math
import sys

import numpy as np

for _p in ("/opt/trn_rl_repo",):
    if _p not in sys.path:
        sys.path.append(_p)

B, L, H, NH, K, NC = 32, 400, 512, 8, 7, 4
D = H // NH          # 64
NCORES = 8
BPC = B // NCORES    # batches per core
G = H // 128         # 4 feature groups
KC = 100             # key chunk of L=400 (4 chunks)
SR = 97              # stats tile rows; batch b's row = 32*b
PAD = K // 2         # 3
LP = L + 8           # padded slot width: y data at col 4
EPS = 1e-5
H3 = 3 * H

_cache = {}


def _bf16(a):
    import ml_dtypes
    return np.asarray(a, np.float32).astype(ml_dtypes.bfloat16)


def _pos_encoding():
    pos = np.arange(L, dtype=np.float64)[:, None]
    div = np.exp(np.arange(0, H, 2, dtype=np.float64) * (-math.log(10000.0) / H))
    pe = np.zeros((L, H), np.float64)
    pe[:, 0::2] = np.sin(pos * div)
    pe[:, 1::2] = np.cos(pos * div)
    return pe.astype(np.float32)


def _install_drain_patch(tile_mod):
    """This walrus build rejects >2 sem waits on one control instruction
    ("Too many sync wait commands"). TileContext's closing drain carries the
    whole global clock; spread it over 1-wait nops instead."""
    from concourse.vector_clock import ScopedClock, VectorClock

    def _patched_drain_and_barrier(self, tick_clock, wait_clock):
        gc = tick_clock.global_clock
        n = len(gc)
        for i in range(n):
            t = gc[i]
            if t > 0:
                nop = self.nc.sync.nop()
                vec = [0] * n
                vec[i] = t
                wait_clock.add_sem_waits(nop.ins, ScopedClock({None: VectorClock(vec)}))
        self.nc.sync.drain()
        self.nc.all_engine_barrier()
        assert self.sems is not None
        popped = self.nc._tile_sem_poison_stack.pop()
        assert popped is self._sem_poison
        self.nc.clear_and_free_semaphores(list(self.sems.allocated().values()))
        self.nc.all_engine_barrier()

    tile_mod.TileContext._drain_and_barrier = _patched_drain_and_barrier


def _split_excess_waits(nc, mybir, limit=1):
    """This walrus build caps semaphore waits per instruction. Move excess
    waits onto same-engine InstNoOp carriers placed just before the
    instruction (engines execute their stream in order, so ordering is
    preserved)."""
    import bass_rust
    for f in nc.m.functions:
        for blk in f.blocks:
            out = []
            changed = False
            for inst in blk.instructions:
                si = inst.sync_info
                waits = list(si.on_wait) if si is not None and si.on_wait else []
                if len(waits) > limit:
                    head, keep = waits[:-limit], waits[-limit:]
                    for j, w in enumerate(head):
                        nop = bass_rust.InstNoOp(
                            name=f"{inst.name}_w{j}", ins=[], outs=[])
                        nop.engine = inst.engine
                        nop.sync_info = mybir.SyncInfo(on_wait=[w],
                                                       on_update=[])
                        nc.register_instruction(nop)
                        out.append(nop)
                    si.on_wait = keep
                    changed = True
                out.append(inst)
            if changed:
                blk.instructions[:] = out


def _build_program(mask_all_ones, repeat=1):
    import concourse.bass as bass
    import concourse.mybir as mybir
    import concourse.tile as tile

    _install_drain_patch(tile)

    f32 = mybir.dt.float32
    f32r = mybir.dt.float32r
    bf16 = mybir.dt.bfloat16
    AF = mybir.ActivationFunctionType
    OP = mybir.AluOpType

    nc = bass.Bass("TRN2", target_bir_lowering=False, debug=False,
                   num_devices=NCORES)

    # ---------------- DRAM parameters (per core) ----------------
    xT = nc.dram_tensor("xT", [BPC, 128, G * L], bf16, kind="ExternalInput").ap()
    wpw = nc.dram_tensor("wpw", [NC, 128, G * H], bf16, kind="ExternalInput").ap()
    dwdiag = nc.dram_tensor("dwdiag", [NC, 128, K * G * 128], bf16,
                            kind="ExternalInput").ap()
    wj = nc.dram_tensor("wj", [K, NC * H], f32r, kind="ExternalInput").ap()
    pwb = nc.dram_tensor("pwb", [128, NC * G], f32, kind="ExternalInput").ap()
    wqkv = nc.dram_tensor("wqkv", [128, G * H3], bf16, kind="ExternalInput").ap()
    csqkv = nc.dram_tensor("csqkv", [2, 2 * H], f32r, kind="ExternalInput").ap()
    csv = nc.dram_tensor("csv", [2, H], f32r, kind="ExternalInput").ap()
    wout = nc.dram_tensor("wout", [128, G * H], bf16, kind="ExternalInput").ap()
    bout = nc.dram_tensor("bout", [128, G], f32, kind="ExternalInput").ap()
    wff = nc.dram_tensor("wff", [128, G * H], bf16, kind="ExternalInput").ap()
    csff = nc.dram_tensor("csff", [1, H], f32r, kind="ExternalInput").ap()
    bff = nc.dram_tensor("bff", [128, G], f32, kind="ExternalInput").ap()
    eb4 = nc.dram_tensor("eb4", [128, BPC * 64], bf16, kind="ExternalInput").ap()
    ones1 = nc.dram_tensor("ones1", [1, 128], f32r, kind="ExternalInput").ap()
    ones1f = nc.dram_tensor("ones1f", [1, 64], f32r, kind="ExternalInput").ap()
    nrowi = nc.dram_tensor("nrowi", [2, LP], f32r, kind="ExternalInput").ap()
    voinit = nc.dram_tensor("voinit", [128, NH * (D + 1)], bf16,
                            kind="ExternalInput").ap()
    mb = nc.dram_tensor("mb", [128, BPC * G], f32, kind="ExternalInput").ap()
    outT = nc.dram_tensor("outT", [BPC, 128, G * L], bf16,
                          kind="ExternalOutput").ap()

    with tile.TileContext(nc) as tc:
        with (
            tc.tile_pool(name="const", bufs=1) as constp,
            tc.tile_pool(name="xp", bufs=1) as xp,
            tc.tile_pool(name="wp", bufs=1) as wp,
            tc.tile_pool(name="scr", bufs=2) as scr,
            tc.tile_pool(name="attp", bufs=1) as attp,
            tc.tile_pool(name="exptp", bufs=2) as exptp,
            tc.tile_pool(name="ps", bufs=2, space="PSUM") as ps,
        ):
            c_zero = constp.tile([128, 1], f32, tag="c_zero")
            nc.vector.memset(c_zero[:], 0.0)
            c_eps = constp.tile([128, 1], f32, tag="c_eps")
            nc.vector.memset(c_eps[:], EPS)
            nc.const_aps.aps[(f32, 0.0)] = c_zero[:]
            nc.const_aps.aps[(f32, EPS)] = c_eps[:]

            def const_tile(shape, src, nm, dt=f32):
                t = constp.tile(shape, dt, tag=nm, name=nm)
                nc.sync.dma_start(t[:], src)
                return t

            # y tiles: one [128, G*LP] padded tile per batch parity; data for
            # group g at cols [g*LP+4, g*LP+4+L). Margins zeroed once.
            y4 = [constp.tile([128, G * LP], bf16, tag=f"y4_{p}",
                              name=f"y4_{p}") for p in range(2)]
            y4o = [constp.tile([128, G * LP], bf16, tag=f"y4o_{p}",
                               name=f"y4o_{p}") for p in range(2)]
            for p in range(2):
                nc.vector.memset(y4[p][:], 0.0)
                nc.vector.memset(y4o[p][:], 0.0)
            # per-batch shift/ones row pairs: row0 = -m*s (data at col 4),
            # row1 = ones over the data region
            nrow2 = [constp.tile([2, LP], f32r, tag=f"nrow2_{b}",
                                 name=f"nrow2_{b}")
                     for b in range(BPC)]
            for b in range(BPC):
                nc.sync.dma_start(nrow2[b][:], nrowi[:])
            # persistent v^T tiles with ones column pre-set
            vo = [[constp.tile([128, NH * (D + 1)], bf16,
                               tag=f"vo_{c}_{p}", name=f"vo_{c}_{p}")
                   for p in range(2)] for c in range(G)]
            for c in range(G):
                for p in range(2):
                    nc.sync.dma_start(vo[c][p][:], voinit[:])

            for _rep in range(repeat):
                # -------- activation loads first, then consts, then weights
                # in use order (single HWDGE queue drains in issue order) ----
                xt = []      # per-batch [128, G*L] residual tiles
                for b in range(BPC):
                    t = xp.tile([128, G * L], bf16, tag=f"x_{b}", name=f"x_{b}")
                    nc.sync.dma_start(t[:], xT[b])
                    xt.append(t)

                c_eb4 = const_tile([128, BPC * 64], eb4[:], 'eb4', bf16)
                c_ones1 = const_tile([1, 128], ones1[:], 'ones1', f32r)
                c_ones1f = const_tile([1, 64], ones1f[:], 'ones1f', f32r)
                c_wj = const_tile([K, NC * H], wj[:], 'wj', f32r)
                c_pwb = const_tile([128, NC * G], pwb[:], 'pwb')

                w_pw_all, w_dg = [], []
                for i in range(NC):
                    t = wp.tile([128, G * H], bf16, tag=f"pw_{i}", name=f"pw_{i}")
                    nc.sync.dma_start(t[:], wpw[i])
                    w_pw_all.append(t)
                    t = wp.tile([128, K * G * 128], bf16, tag=f"dg_{i}",
                                name=f"dg_{i}")
                    nc.sync.dma_start(t[:], dwdiag[i])
                    w_dg.append(t)

                w_qkv = wp.tile([128, G * H3], bf16, tag="qkv", name="qkv")
                nc.sync.dma_start(w_qkv[:], wqkv[:])
                c_csqkv = const_tile([2, 2 * H], csqkv[:], 'csqkv', f32r)
                c_csv = const_tile([2, H], csv[:], 'csv', f32r)
                w_out = wp.tile([128, G * H], bf16, tag="out", name="out")
                nc.sync.dma_start(w_out[:], wout[:])
                c_bout = const_tile([128, G], bout[:], 'bout')
                w_ff = wp.tile([128, G * H], bf16, tag="ff", name="ff")
                nc.sync.dma_start(w_ff[:], wff[:])
                c_csff = const_tile([1, H], csff[:], 'csff', f32r)
                c_bff = const_tile([128, G], bff[:], 'bff')
                c_mb = None if mask_all_ones else const_tile(
                    [128, BPC * G], mb[:], 'mb')

                def xsl(b, g):
                    return xt[b][:, L * g:L * (g + 1)]

                def ysl(p, g, j):
                    """y view of group g shifted for tap j (x-hat[l+j-3]),
                    always 4-byte aligned: odd j reads y4 (data at col 4),
                    even j reads the one-right-shifted copy y4o (col 5)."""
                    if j % 2 == 1:
                        return y4[p][:, LP * g + j + 1:LP * g + j + 1 + L]
                    return y4o[p][:, LP * g + j + 2:LP * g + j + 2 + L]

                # ---------- layer norm ----------
                def layer_norm(tag, consume, sq_dve=False):
                    """LN over H, batch-pipelined: each batch gets its own
                    [2, L] stats PSUM (row0 = E[x], row1 = E[x^2]) so batch
                    b's consume starts as soon as its own 8 stats matmuls
                    and short fp32 chain finish. ACT Exp writes the bf16 s
                    row directly; the shift row -m*s lands straight in
                    nrow2[b]."""
                    with tc.tile_pool(name=f"pst_{tag}", bufs=1,
                                      space="PSUM") as pst:
                        for b in range(BPC):
                            p = b % 2
                            ebT = c_eb4[:, 64 * b:64 * (b + 1)]
                            ex1 = pst.tile([64, L], f32, tag="st1",
                                           name=f"ex1_{tag}_{b}")
                            ex2 = pst.tile([64, L], f32, tag="st2",
                                           name=f"ex2_{tag}_{b}")
                            for g in range(G):
                                nc.tensor.matmul(ex1[:], ebT, xsl(b, g),
                                                 start=(g == 0),
                                                 stop=(g == G - 1))
                            sq = scr.tile([128, G * L], bf16, tag="sq",
                                          name=f"sq_{tag}_{b}")
                            if sq_dve:
                                nc.vector.tensor_tensor(sq[:], xt[b][:],
                                                        xt[b][:], op=OP.mult)
                            else:
                                nc.scalar.activation(sq[:], xt[b][:], AF.Square)
                            for g in range(G):
                                nc.tensor.matmul(ex2[:], ebT,
                                                 sq[:, L * g:L * (g + 1)],
                                                 start=(g == 0),
                                                 stop=(g == G - 1))
                            # (DVE may read only one PSUM operand per op)
                            m_sb = scr.tile([1, L], f32, tag="ln_m", bufs=3,
                                            name=f"m_{tag}_{b}")
                            nc.scalar.activation(m_sb[:], ex1[0:1, :],
                                                 AF.Copy)
                            msq = scr.tile([1, L], f32, tag="ln_tmp", bufs=3,
                                           name=f"msq_{tag}_{b}")
                            nc.vector.tensor_tensor(msq[:], m_sb[:],
                                                    ex1[0:1, :], op=OP.mult)
                            var = scr.tile([1, L], f32, tag="ln_tmp", bufs=3,
                                           name=f"var_{tag}_{b}")
                            nc.vector.scalar_tensor_tensor(
                                var[:], msq[:], -1.0, ex2[0:1, :],
                                op0=OP.mult, op1=OP.add)
                            lnv = scr.tile([1, L], f32, tag="ln_tmp", bufs=3,
                                           name=f"lnv_{tag}_{b}")
                            nc.scalar.activation(lnv[:], var[:], AF.Ln, bias=EPS)
                            s1 = scr.tile([1, L], f32r, tag="ln_s1", bufs=4,
                                          name=f"s1_{tag}_{b}")
                            nc.scalar.activation(s1[:], lnv[:], AF.Exp,
                                                 scale=-0.5)
                            nc.vector.scalar_tensor_tensor(
                                nrow2[b][0:1, 4:4 + L], m_sb[:], -1.0, s1[:],
                                op0=OP.mult, op1=OP.mult)
                            sbp = ps.tile([128, L], f32, tag="mm",
                                          name=f"sb_{tag}_{b}")
                            nc.tensor.matmul(sbp[:], c_ones1[:], s1[:],
                                             start=True, stop=True)
                            sb = scr.tile([128, L], bf16, tag="sb", bufs=2,
                                          name=f"sbs_{tag}_{b}")
                            nc.scalar.activation(sb[:], sbp[:], AF.Copy)
                            for g in range(G):
                                nc.vector.tensor_tensor(
                                    y4[p][:, LP * g + 4:LP * g + 4 + L],
                                    xsl(b, g), sb[:], op=OP.mult)
                            consume(b, p)

                # ================= conv blocks =================
                for i in range(NC):
                    def conv_consume(b, p, i=i):
                        # one-right-shifted copy of y for the even taps
                        nc.sync.dma_start(
                            y4o[p][:].rearrange("q (g l) -> q g l", l=LP)
                            [:, :, 5:5 + L],
                            y4[p][:].rearrange("q (g l) -> q g l", l=LP)
                            [:, :, 4:4 + L])
                        # shifted copies of the -m*s row for the K=7 shift-
                        # correction matmul (reads padded source)
                        nt = scr.tile([K, L], f32r, tag="nshift", bufs=2,
                                      name=f"nt_{i}_{b}")
                        for j in range(K):
                            nc.sync.dma_start(nt[j:j + 1, :],
                                              nrow2[b][0:1, j + 1:j + 1 + L])
                        acc = scr.tile([128, G * L], bf16, tag="acc", bufs=2,
                                       name=f"acc_{i}_{b}")
                        with tc.tile_pool(name=f"ptap_{i}_{b}", bufs=2,
                                          space="PSUM") as ptap, \
                             tc.tile_pool(name=f"pmc_{i}_{b}", bufs=2,
                                          space="PSUM") as pmc:
                            for g in range(G):
                                pt = ptap.tile([128, L], f32, tag="tap",
                                               name=f"pt_{i}_{b}_{g}")
                                jorder = (1, 3, 5, 0, 2, 4, 6)
                                for ji, j in enumerate(jorder):
                                    nc.tensor.matmul(
                                        pt[:],
                                        w_dg[i][:, (j * G + g) * 128:
                                                (j * G + g + 1) * 128],
                                        ysl(p, g, j),
                                        start=(ji == 0), stop=(ji == K - 1))
                                nc.scalar.activation(
                                    acc[:, L * g:L * (g + 1)], pt[:], AF.Copy)
                            cvall = scr.tile([128, G * L], bf16, tag="cv",
                                             name=f"cv_{i}_{b}")
                            for o in range(G):
                                pm = pmc.tile([128, L], f32, tag="pm",
                                              name=f"pm_{i}_{b}_{o}")
                                for g in range(G):
                                    nc.tensor.matmul(
                                        pm[:],
                                        w_pw_all[i][:, H * g + 128 * o:
                                                    H * g + 128 * (o + 1)],
                                        acc[:, L * g:L * (g + 1)],
                                        start=(g == 0), stop=False)
                                nc.tensor.matmul(
                                    pm[:],
                                    c_wj[:, i * H + 128 * o:i * H + 128 * (o + 1)],
                                    nt[:], start=False, stop=True)
                                nc.scalar.activation(
                                    cvall[:, L * o:L * (o + 1)], pm[:], AF.Relu,
                                    bias=c_pwb[:, i * G + o:i * G + o + 1])
                            nc.vector.tensor_tensor(xt[b][:], xt[b][:],
                                                    cvall[:], op=OP.add)

                    layer_norm(f"c{i}", conv_consume)

                # ================= attention =================
                def att_consume(b, p):
                    qsb, ksb = [], []
                    for m in range(2 * G):
                        pm = ps.tile([128, L], f32, tag="mm", name=f"pqk_{b}_{m}")
                        for g in range(G):
                            nc.tensor.matmul(
                                pm[:],
                                w_qkv[:, H3 * g + 128 * m:H3 * g + 128 * (m + 1)],
                                y4[p][:, LP * g + 4:LP * g + 4 + L],
                                start=(g == 0), stop=False)
                        nc.tensor.matmul(
                            pm[:], c_csqkv[:, 128 * m:128 * (m + 1)],
                            nrow2[b][0:2, 4:4 + L], start=False, stop=True)
                        t = attp.tile([128, L], bf16, tag=f"qk_{m}",
                                      bufs=2, name=f"qk_{b}_{m}")
                        nc.scalar.activation(t[:], pm[:], AF.Copy)
                        (qsb if m < G else ksb).append(t)
                    # v pre-transposed with bias+shift via K=2 row pair;
                    # ones column pre-set gives sum(exp) free in AV
                    for c in range(G):
                        pv = ps.tile([128, H], f32, tag="mm", name=f"pv_{b}_{c}")
                        for g in range(G):
                            nc.tensor.matmul(
                                pv[0:KC, :],
                                y4[p][:, LP * g + 4 + KC * c:
                                      LP * g + 4 + KC * (c + 1)],
                                w_qkv[:, H3 * g + 2 * H:H3 * g + 3 * H],
                                start=(g == 0), stop=False)
                        nc.tensor.matmul(
                            pv[0:KC, :],
                            nrow2[b][0:2, 4 + KC * c:4 + KC * (c + 1)],
                            c_csv[:], start=False, stop=True)
                        vv = vo[c][p][:].rearrange("p (h d) -> p h d", d=D + 1)
                        pvv = pv[:, 0:H].rearrange("p (h d) -> p h d", d=D)
                        nc.scalar.activation(vv[0:KC, :, 0:D], pvv[0:KC, :, :],
                                             AF.Copy)
                    att = []
                    for g in range(G):
                        att.append(attp.tile([128, L], bf16, tag=f"ao_{g}",
                                             bufs=2, name=f"ao_{b}_{g}"))
                    with tc.tile_pool(name=f"psatt_{b}", bufs=2,
                                      space="PSUM") as psatt:
                        for h in range(NH):
                            g2, ho = h // 2, (h % 2) * 64
                            # scores^T [k, q], one 1-bank psum per key chunk
                            ex = exptp.tile([128, G * L], bf16, tag="expT",
                                            bufs=3, name=f"ex_{b}_{h}")
                            for c in range(G):
                                sc = psatt.tile([128, 512], f32, tag="sc",
                                                name=f"sc_{b}_{h}_{c}")
                                nc.tensor.matmul(
                                    sc[0:KC, 0:L],
                                    ksb[g2][ho:ho + 64, KC * c:KC * (c + 1)],
                                    qsb[g2][ho:ho + 64, :],
                                    start=True, stop=True)
                                if mask_all_ones:
                                    nc.scalar.activation(
                                        ex[0:KC, L * c:L * c + L],
                                        sc[0:KC, 0:L], AF.Exp)
                                else:
                                    nc.scalar.activation(
                                        ex[0:KC, L * c:L * c + L],
                                        sc[0:KC, 0:L], AF.Exp,
                                        bias=c_mb[0:KC,
                                                  b * G + c:b * G + c + 1])
                            # AV with ones row -> [65, L]; row 64 = sum(exp)
                            av = psatt.tile([D + 1, L], f32, tag="av",
                                            name=f"av_{b}_{h}")
                            for c in range(G):
                                nc.tensor.matmul(
                                    av[:],
                                    vo[c][p][0:KC,
                                             (D + 1) * h:(D + 1) * (h + 1)],
                                    ex[0:KC, L * c:L * c + L],
                                    start=(c == 0), stop=(c == G - 1))
                            inv = scr.tile([1, L], f32r, tag="inv",
                                           name=f"inv_{b}_{h}")
                            with nc.allow_low_precision(reason="f32r feed"):
                                nc.vector.reciprocal(inv[:], av[D:D + 1, :])
                            # inv broadcast rides the sc tag rotation (slot
                            # freed once this head's exps are read)
                            ib = psatt.tile([64, 512], f32, tag="sc",
                                            name=f"ib_{b}_{h}")
                            nc.tensor.matmul(ib[:, 0:L], c_ones1f[:], inv[:],
                                             start=True, stop=True)
                            ibs = scr.tile([64, L], bf16, tag="ibs",
                                           name=f"ibs_{b}_{h}")
                            nc.scalar.activation(ibs[:], ib[:, 0:L], AF.Copy)
                            nc.vector.tensor_tensor(att[g2][ho:ho + 64, :],
                                                    av[0:D, :], ibs[:],
                                                    op=OP.mult)
                    # output projection + residual for this batch
                    aoall = scr.tile([128, G * L], bf16, tag="cv",
                                     name=f"aa_{b}")
                    for o in range(G):
                        pm = ps.tile([128, L], f32, tag="mm", name=f"po_{b}_{o}")
                        for g in range(G):
                            nc.tensor.matmul(
                                pm[:],
                                w_out[:, H * g + 128 * o:H * g + 128 * (o + 1)],
                                att[g][:], start=(g == 0), stop=(g == G - 1))
                        nc.vector.tensor_scalar_add(
                            aoall[:, L * o:L * (o + 1)], pm[:],
                            c_bout[:, o:o + 1])
                    nc.vector.tensor_tensor(xt[b][:], xt[b][:], aoall[:],
                                            op=OP.add)

                layer_norm("a", att_consume, sq_dve=True)

                # ================= feed-forward =================
                def ff_consume(b, p):
                    foall = scr.tile([128, G * L], bf16, tag="cv",
                                     name=f"fo_{b}")
                    for o in range(G):
                        pm = ps.tile([128, L], f32, tag="mm", name=f"pff_{b}_{o}")
                        for g in range(G):
                            nc.tensor.matmul(
                                pm[:],
                                w_ff[:, H * g + 128 * o:H * g + 128 * (o + 1)],
                                y4[p][:, LP * g + 4:LP * g + 4 + L],
                                start=(g == 0), stop=False)
                        nc.tensor.matmul(
                            pm[:], c_csff[:, 128 * o:128 * (o + 1)],
                            nrow2[b][0:1, 4:4 + L], start=False, stop=True)
                        nc.scalar.activation(foall[:, L * o:L * (o + 1)], pm[:],
                                             AF.Relu, bias=c_bff[:, o:o + 1])
                    # final residual add (bf16 store; host converts to f32)
                    xo = scr.tile([128, G * L], bf16, tag="xo", bufs=2,
                                  name=f"xo_{b}")
                    nc.vector.tensor_tensor(xo[:], xt[b][:], foall[:], op=OP.add)
                    nc.sync.dma_start(outT[b], xo[:])

                layer_norm("f", ff_consume, sq_dve=True)
    _split_excess_waits(nc, mybir, limit=1)
    return nc


def _prep_inputs(inputs):
    """Host-side packing shared by all cores."""
    x = np.asarray(inputs["x"], np.float32)
    mask = np.asarray(inputs["mask"])
    pe = _pos_encoding()
    xpe = np.ascontiguousarray((x + pe[None]).transpose(0, 2, 1))  # (B, H, L)

    conv_ln_g = np.asarray(inputs["conv_ln_g"], np.float32)
    conv_ln_b = np.asarray(inputs["conv_ln_b"], np.float32)
    dw_w = np.asarray(inputs["dw_w"], np.float32)[:, :, 0, :]  # (NC, H, K)
    dw_b = np.asarray(inputs["dw_b"], np.float32)
    pw_w = np.asarray(inputs["pw_w"], np.float32)        # (NC, H_out, H_in)
    pw_b = np.asarray(inputs["pw_b"], np.float32)
    att_ln_g = np.asarray(inputs["att_ln_g"], np.float32)
    att_ln_b = np.asarray(inputs["att_ln_b"], np.float32)
    qkv_w = np.asarray(inputs["qkv_w"], np.float32)      # (3H, H)
    qkv_b = np.asarray(inputs["qkv_b"], np.float32)
    out_w = np.asarray(inputs["out_w"], np.float32)
    out_b = np.asarray(inputs["out_b"], np.float32)
    ff_ln_g = np.asarray(inputs["ff_ln_g"], np.float32)
    ff_ln_b = np.asarray(inputs["ff_ln_b"], np.float32)
    ff_w = np.asarray(inputs["ff_w"], np.float32)
    ff_b = np.asarray(inputs["ff_b"], np.float32)

    def vecpack(v):  # (..., H) -> [128, n*G], col n*G-index = (outer, g)
        v = np.asarray(v, np.float32).reshape(-1, G, 128)
        return np.ascontiguousarray(v.transpose(2, 0, 1).reshape(128, -1))

    # ---- conv blocks: fold LN gamma into dw weights, beta into pw bias ----
    dwp = dw_w * conv_ln_g[:, :, None]                    # (NC, H, K) folded
    const_c = conv_ln_b * dw_w.sum(-1)                    # beta * sum_j dw
    eff_pwb = pw_b + np.einsum("ioc,ic->io", pw_w, dw_b + const_c)
    # K=7 shift-correction weights: WJ[i][j, o] = sum_c pw[o,c]*dwp[c,j]
    wj_all = np.einsum("ioc,icj->ijo", pw_w, dwp)         # (NC, K, H)
    wj_pack = np.ascontiguousarray(
        wj_all.transpose(1, 0, 2).reshape(K, NC * H))
    # diag matrices for the PE taps, pre-packed to the SBUF tile layout:
    # [i, p, (j*G+g)*128 + c] = dwp[i, 128g+p, j] if c == p else 0
    dwdiag = np.zeros((NC, 128, K * G * 128), np.float32)
    for i in range(NC):
        for j in range(K):
            for g in range(G):
                blk = (j * G + g) * 128
                dwdiag[i, np.arange(128), blk + np.arange(128)] = \
                    dwp[i, 128 * g:128 * (g + 1), j]

    # ---- attention: fold gamma into W rows, beta into bias; scale q ----
    scale = 1.0 / math.sqrt(D)
    wq = qkv_w * att_ln_g[None, :]
    bq = qkv_b + qkv_w @ att_ln_b
    wq[:H] *= scale
    bq[:H] *= scale
    cs = wq.sum(1)                                        # (3H,)
    csqkv = np.stack([cs[:2 * H], bq[:2 * H]], 0)          # (2, 2H)
    csv = np.stack([cs[2 * H:], bq[2 * H:]], 0)            # (2, H)

    # ---- ff: fold gamma/beta ----
    wf = ff_w * ff_ln_g[None, :]
    bf_ = ff_b + ff_w @ ff_ln_b
    csff = wf.sum(1)[None, :]                             # (1, H)

    eb4 = np.zeros((128, BPC * 64), np.float32)
    eb4[:, 0::64] = 1.0 / H

    def sbpack(w):
        # (H_in, N) -> [128, G*N]: partition p, col g*N+n = w[128g+p, n]
        n = w.shape[1]
        return np.ascontiguousarray(
            w.reshape(G, 128, n).transpose(1, 0, 2).reshape(128, G * n))

    common = {
        "wpw": _bf16(np.stack([sbpack(pw_w[i].T) for i in range(NC)])),
        "dwdiag": _bf16(dwdiag),
        "wj": wj_pack,
        "pwb": vecpack(eff_pwb),
        "wqkv": _bf16(sbpack(wq.T)),
        "csqkv": np.ascontiguousarray(csqkv),
        "csv": np.ascontiguousarray(csv),
        "wout": _bf16(sbpack(out_w.T)),
        "bout": vecpack(out_b),
        "wff": _bf16(sbpack(wf.T)),
        "csff": np.ascontiguousarray(csff),
        "bff": vecpack(bf_),
        "eb4": _bf16(eb4),
        "ones1": np.ones((1, 128), np.float32),
        "ones1f": np.ones((1, 64), np.float32),
        "voinit": _bf16(np.tile(
            np.concatenate([np.zeros((1, NH, D), np.float32),
                            np.ones((1, NH, 1), np.float32)],
                           2).reshape(1, NH * (D + 1)), (128, 1))),
        "nrowi": np.ascontiguousarray(np.concatenate(
            [np.zeros((1, LP), np.float32),
             np.pad(np.ones((1, L), np.float32),
                    ((0, 0), (4, LP - L - 4)))], 0)),
    }
    mask_all_ones = bool((np.asarray(mask) > 0).all())
    mbs = []
    for c in range(NCORES):
        mbc = np.zeros((128, BPC * G), np.float32)
        for b in range(BPC):
            mrow = np.asarray(mask[c * BPC + b])
            for ch in range(G):
                mbc[0:KC, b * G + ch] = np.where(
                    mrow[KC * ch:KC * (ch + 1)] > 0, 0.0, -1e9)
        mbs.append(mbc)
    # (B, H, L) -> per-batch [128, G*L] with group g at cols [g*L, (g+1)*L)
    xp8 = np.ascontiguousarray(
        xpe.reshape(B, G, 128, L).transpose(0, 2, 1, 3).reshape(B, 128, G * L))
    x_shards = [_bf16(xp8[c * BPC:(c + 1) * BPC]) for c in range(NCORES)]
    return common, x_shards, mbs, mask_all_ones


def get_program(mask_all_ones=True, repeat=1):
    key = ("prog", mask_all_ones, repeat)
    if key not in _cache:
        _cache[key] = _build_program(mask_all_ones, repeat)
    return _cache[key]


def make_in_maps(inputs):
    common, x_shards, mbs, mask_all_ones = _prep_inputs(inputs)
    in_maps = []
    for c in range(NCORES):
        m = dict(common)
        m["xT"] = x_shards[c]
        m["mb"] = mbs[c]
        in_maps.append(m)
    return in_maps, mask_all_ones


def gather_output(results):
    outs = []
    for c in range(NCORES):
        r = np.asarray(results[c]["outT"], np.float32)   # [BPC, 128, G*L]
        r = r.reshape(BPC, 128, G, L).transpose(0, 3, 2, 1).reshape(BPC, L, H)
        outs.append(r)
    return np.ascontiguousarray(np.concatenate(outs, axis=0), dtype=np.float32)


def kernel(**inputs):
    from concourse.bass_utils import run_bass_kernel_spmd

    in_maps, mask_all_ones = make_in_maps(inputs)
    nc = get_program(mask_all_ones)
    res = run_bass_kernel_spmd(nc, in_maps, list(range(NCORES)))
    return gather_output(res.results)
